# revision 23
# baseline (speedup 1.0000x reference)
"""Trainium2 Bass kernel for nn_Bottleneck_refine (masked grouped bottleneck).

Sharding: data-parallel over batch B=8 across 8 NeuronCores (1 sample/core).

Math factorization (validated vs reference):
  - All eval-mode BNs folded into conv weights/biases on host.
  - mask = (wd/64 . pooled_sum + bd + gumbel_diff) > 0   (hard gumbel forward
    value is exactly the one-hot argmax; softmax is monotone).
  - x*mask1 is algebraically removable: conv1 is 1x1 grouped and the mask2
    multiply after relu(bn1(.)) re-zeroes exactly the masked cells.
  - Spatial layout on chip: flat padded rows of width 57 (56 cols + one zero
    separator) x 58 rows (zero top/bottom rows) + 1 leading guard col, so a
    3x3 conv tap (dh,dw) is a flat column shift dh*57+dw.
  - conv_gs (1024->16, 3x3) is done as two tap-packed matmul passes
    (M = taps x 16 out-ch: 5 taps then 4 taps) producing per-tap partials,
    then 9 selector-matmuls accumulate the shifted tap-sum in PSUM.
  - conv1/residual-identity in float32r (full-rate fp32 matmul), conv2/conv3
    and selector passes in bf16 (intermediates only; residual path stays f32).
"""

import os
import sys

import numpy as np

for _p in ("/opt/trn_rl_repo",):
    if os.path.isdir(_p) and _p not in sys.path:
        sys.path.insert(0, _p)

import ml_dtypes  # noqa: E402

BF16 = ml_dtypes.bfloat16

# problem constants
B = 8
CIN = 1024
H = W = 56
G = 4
MID = 256
PLANES = 1024
MS = 7
EPS = 1e-5

# padded flat spatial layout
WP = 57            # 56 data cols + 1 zero separator
HP = 58            # zero row 0, data rows 1..56, zero row 57
NPAD = HP * WP     # 3306
BUFW = NPAD + 2    # guard cols at both ends (flat index -1 / +NPAD of tap shifts)
NPIX = H * W       # 3136

TAPS = [(dh, dw) for dh in (-1, 0, 1) for dw in (-1, 0, 1)]
DELTA = [dh * WP + dw for dh, dw in TAPS]
NCTAPS = [0, 1, 2, 3, 5, 6, 7, 8]   # non-center taps (center=4 applied on x)
PAIRS = [(0, 1), (2, 3), (4, 5), (6,)]

NCHUNK_A = [472, 472, 472, 472, 472, 472, 474]   # setA passes cover [0, 3306)
CHUNKS = 7                                        # interior chunks of 8 rows


def _fold(w, g, b, m, v):
    s = (np.asarray(g, np.float32) / np.sqrt(np.asarray(v, np.float32) + EPS))
    w = np.asarray(w, np.float32)
    return (w * s[:, None, None, None]).astype(np.float32), \
        (np.asarray(b, np.float32) - np.asarray(m, np.float32) * s).astype(np.float32)


def _host_prep(inputs):
    """Build all host-side arrays (per-core and shared)."""
    x = np.asarray(inputs["x"], np.float32)
    wgs, bgs = _fold(inputs["conv_gs_w"], inputs["bn_gs_g"], inputs["bn_gs_b"],
                     inputs["bn_gs_m"], inputs["bn_gs_v"])
    w1, b1 = _fold(inputs["conv1_w"], inputs["bn1_g"], inputs["bn1_b"],
                   inputs["bn1_m"], inputs["bn1_v"])
    w2, b2 = _fold(inputs["conv2_w"], inputs["bn2_g"], inputs["bn2_b"],
                   inputs["bn2_m"], inputs["bn2_v"])
    w3, b3 = _fold(inputs["conv3_w"], inputs["bn3_g"], inputs["bn3_b"],
                   inputs["bn3_m"], inputs["bn3_v"])

    # ---- x: fp16 cell-aligned column tiles [7, CIN, 456] per sample
    # (image rows 8c..8c+7 with one zero separator col per row)
    xp = np.zeros((B, CIN, 56, WP), np.float16)
    xp[:, :, :, 0:56] = x.astype(np.float16)
    xcol = np.transpose(xp.reshape(B, CIN, 7, 456), (0, 2, 1, 3)).copy()

    # ---- conv_gs tap-packed lhsT:  wA1 [8,128,80], wA2 [8,128,64]
    # col ti*16+oc of set s = tap SETs[ti], out-ch oc; rows = global in-ch.
    wgs9 = wgs.reshape(16, 256, 9)                      # [oc, icl, tap]
    wA = np.zeros((8, 128, 9, 16), np.float32)          # [kchunk, r, tap, oc]
    for k in range(8):
        ic = 128 * k + np.arange(128)
        for oc in range(16):
            m = (ic // 256) == (oc // 4)
            wA[k, m, :, oc] = wgs9[oc, ic[m] % 256, :]
    # taps 0..3,5..8 packed into M=128 (center tap 4 applied directly on x)
    wA8 = wA[:, :, NCTAPS, :].reshape(8, 128, 128).copy()
    wct = wA[:, :, 4, :].copy()                          # [8,128,16]

    # ---- selector lhsT: sel8[ti] picks tap NCTAPS[ti]'s 16 rows
    sel8 = np.zeros((8, 128, 16), np.float32)
    for ti in range(8):
        sel8[ti, ti * 16 + np.arange(16), np.arange(16)] = 1.0

    # ---- conv1 lhsT [2,4,128,128] (pairset p, kchunk j): rows = in-ch
    # 512p+128j+r, cols mm = out-ch 128p+mm; nonzero iff j//2 == mm//64.
    w1m = w1[:, :, 0, 0]                                # [256 out, 256 in-local]
    w1T = np.zeros((2, 4, 128, 128), np.float32)
    for p in range(2):
        for j in range(4):
            for mm in range(128):
                if j // 2 != mm // 64:
                    continue
                w1T[p, j, :, mm] = w1m[128 * p + mm, 128 * (j % 2):128 * (j % 2) + 128]

    # ---- conv2 lhsT [2,9,128,128] bf16 (pairset p, tap t); block-diag 64x64
    w29 = w2.reshape(256, 64, 9)
    w2T = np.zeros((2, 9, 128, 128), np.float32)
    for p in range(2):
        for mm in range(128):
            base = (mm // 64) * 64
            w2T[p, :, base:base + 64, mm] = w29[128 * p + mm].T
    # ---- conv3 lhsT [8,128,128] bf16 (mchunk m8): rows = z2-chunk partition
    # (only the group's 64 rows are nonzero, so base partitions match)
    w3m = w3[:, :, 0, 0]                                # [1024, 64]
    w3T = np.zeros((8, 128, 128), np.float32)
    for m8 in range(8):
        r0 = ((m8 // 2) % 2) * 64
        w3T[m8, r0:r0 + 64, :] = w3m.reshape(8, 128, 64)[m8].T

    ident = np.eye(128, dtype=np.float32)

    # ---- mask decision
    fcw = np.asarray(inputs["fc_gs_w"], np.float32)[:, :, 0, 0]  # [8,4]
    fcb = np.asarray(inputs["fc_gs_b"], np.float32)              # [8]
    L = np.zeros((16, 8), np.float32)
    for c in range(8):
        gconv = c // 2
        L[gconv * 4:(gconv + 1) * 4, c] = fcw[c]
    wd4 = np.zeros((16, 4), np.float32)
    bd = np.zeros(4, np.float32)
    for g in range(4):
        wd4[:, g] = (L[:, 4 + g] - L[:, g]) / 64.0
        bd[g] = fcb[4 + g] - fcb[g]
    gn = np.asarray(inputs["gumbel_noise"], np.float32)
    gd = (gn[:, 1] - gn[:, 0]).reshape(B, 4, 49) + bd[None, :, None]  # [B,4,49]

    # ---- mask expand selector E [2,4,128]
    Emat = np.zeros((2, 4, 128), np.float32)
    for cc in range(2):
        c = np.arange(128)
        Emat[cc, 2 * cc + c // 64, c] = 1.0

    F16 = np.float16
    shared = {
        "wA8": wA8.astype(F16), "wct": wct.astype(F16),
        "sel8": sel8.astype(F16),
        "w1T": w1T.astype(F16),
        "w2T": w2T.astype(F16),
        "w3T": w3T.astype(F16),
        "ident": ident.astype(F16),
        "Emat": Emat,
        "wd4": wd4,
        "bgs": bgs.reshape(16, 1),
        "b1": b1.reshape(2, 128, 1),
        "b2": b2.reshape(2, 128, 1),
        "b3": b3.reshape(8, 128, 1),
    }
    percore = [{"xcol": xcol[s], "gd": gd[s]} for s in range(B)]
    return shared, percore


# ---------------------------------------------------------------------------
# Bass program
# ---------------------------------------------------------------------------

_PROG = None


def _build_program(nsamp=B):
    from contextlib import ExitStack

    import concourse.bacc as bacc
    import concourse.mybir as mybir
    import concourse.tile as tile

    dt = mybir.dt
    F32, F16 = dt.float32, dt.float16
    AF = mybir.ActivationFunctionType
    ALU = mybir.AluOpType

    nc = bacc.Bacc("TRN2", target_bir_lowering=False, debug=False)

    d_x = nc.dram_tensor("xcol", [7, CIN, 456], F16, kind="ExternalInput")
    d_gd = nc.dram_tensor("gd", [4, 49], F32, kind="ExternalInput")
    d_wA8 = nc.dram_tensor("wA8", [8, 128, 128], F16, kind="ExternalInput")
    d_wct = nc.dram_tensor("wct", [8, 128, 16], F16, kind="ExternalInput")
    d_sel8 = nc.dram_tensor("sel8", [8, 128, 16], F16, kind="ExternalInput")
    d_w1 = nc.dram_tensor("w1T", [2, 4, 128, 128], F16, kind="ExternalInput")
    d_w2 = nc.dram_tensor("w2T", [2, 9, 128, 128], F16, kind="ExternalInput")
    d_w3 = nc.dram_tensor("w3T", [8, 128, 128], F16, kind="ExternalInput")
    d_id = nc.dram_tensor("ident", [128, 128], F16, kind="ExternalInput")
    d_E = nc.dram_tensor("Emat", [2, 4, 128], F32, kind="ExternalInput")
    d_wd4 = nc.dram_tensor("wd4", [16, 4], F32, kind="ExternalInput")
    d_bgs = nc.dram_tensor("bgs", [16, 1], F32, kind="ExternalInput")
    d_b1 = nc.dram_tensor("b1", [2, 128, 1], F32, kind="ExternalInput")
    d_b2 = nc.dram_tensor("b2", [2, 128, 1], F32, kind="ExternalInput")
    d_b3 = nc.dram_tensor("b3", [8, 128, 1], F32, kind="ExternalInput")
    d_out = nc.dram_tensor("out", [PLANES, NPIX], F32, kind="ExternalOutput")
    d_mask = nc.dram_tensor("mask_out", [4, 49], F32, kind="ExternalOutput")

    def interior(ap, w=WP):
        """[P, n*w] contiguous slice -> [P, n, 56] strided interior view."""
        return ap.rearrange("p (r w) -> p r w", w=w)[:, :, 0:56]

    def ps_interior(ps, nb):
        """[128,1024] psum pair tile -> [P, nb, 8, 56] interior of bank regions."""
        return (ps[:, 0:512 * nb].rearrange("p (b q) -> p b q", q=512)
                [:, :, 0:456].rearrange("p b (r w) -> p b r w", w=57)[:, :, :, 0:56])

    def ps_regions(ps, nb):
        """[128,1024] psum pair tile -> [P, nb, 456] bank regions."""
        return (ps[:, 0:512 * nb].rearrange("p (b q) -> p b q", q=512)[:, :, 0:456])

    with tile.TileContext(nc) as tc, ExitStack() as ex:
        wpool = ex.enter_context(tc.tile_pool(name="w", bufs=1))
        big = ex.enter_context(tc.tile_pool(name="big", bufs=1))
        ppool = ex.enter_context(tc.tile_pool(name="ps", bufs=4, space="PSUM"))
        spool = ex.enter_context(tc.tile_pool(name="stage", bufs=8))

        def load(dram_ap, shape, dtyp, tag):
            t = wpool.tile(shape, dtyp, tag=tag, name=tag)
            nc.sync.dma_start(t[:], dram_ap)
            return t

        wA8 = [load(d_wA8.ap()[k], [128, 128], F16, f"wA8_{k}") for k in range(8)]
        wct = [load(d_wct.ap()[k], [128, 16], F16, f"wct_{k}") for k in range(8)]
        sel8 = [load(d_sel8.ap()[t], [128, 16], F16, f"sel8_{t}") for t in range(8)]
        w1 = [[load(d_w1.ap()[p, j], [128, 128], F16, f"w1_{p}_{j}")
               for j in range(4)] for p in range(2)]
        w2 = [[load(d_w2.ap()[p, t], [128, 128], F16, f"w2_{p}_{t}")
               for t in range(9)] for p in range(2)]
        w3 = [load(d_w3.ap()[m], [128, 128], F16, f"w3_{m}") for m in range(8)]
        ident = load(d_id.ap()[:], [128, 128], F16, "ident")
        Emat = [load(d_E.ap()[cc], [4, 128], F32, f"E_{cc}") for cc in range(2)]
        wd4 = load(d_wd4.ap()[:], [16, 4], F32, "wd4")
        bgs = load(d_bgs.ap()[:], [16, 1], F32, "bgs")
        b1 = [load(d_b1.ap()[p], [128, 1], F32, f"b1_{p}") for p in range(2)]
        b2 = [load(d_b2.ap()[p], [128, 1], F32, f"b2_{p}") for p in range(2)]
        b3 = [load(d_b3.ap()[m], [128, 1], F32, f"b3_{m}") for m in range(8)]

        a1 = [big.tile([128, BUFW], F16, tag=f"a1_{p}", name=f"a1_{p}") for p in range(2)]
        yA = big.tile([128, BUFW], F16, tag="yA", name="yA")
        z2 = [big.tile([128, 3192], F16, tag=f"z2_{p}", name=f"z2_{p}") for p in range(2)]
        g1 = big.tile([16, NPIX], F16, tag="g1", name="g1")
        p1tmp = big.tile([16, 392], F32, tag="p1tmp", name="p1tmp")
        pooled = big.tile([16, 49], F32, tag="pooled", name="pooled")
        gd_t = big.tile([4, 49], F32, tag="gd", name="gd")
        s_t = big.tile([4, 49], F32, tag="s_t", name="s_t")
        mask = big.tile([4, 49], F32, tag="mask", name="mask")
        mexp = [big.tile([128, 49], F16, tag=f"mexp_{cc}", name=f"mexp_{cc}") for cc in range(2)]

        # zero pad/guard structure once (interior writes preserve it)
        for t in a1 + [yA] + z2:
            nc.vector.memset(t[:], 0.0)

        def chunk_base(c):
            # first flat index (incl. guard offset) of interior chunk c's rows
            return 1 + WP * (1 + 8 * c)

        def mask_bcast(p, c):
            # mask for ch-chunk p at cell-row c, broadcast to [128, 8, 7, 8]
            return (mexp[p][:, 7 * c:7 * c + 7]
                    .unsqueeze(1).unsqueeze(3).broadcast_to([128, 8, 7, 8]))

        for s in range(nsamp):
            # double-buffered x tiles: next sample's loads have no WAR on this
            # sample's reads, so they prefetch across the whole sample
            xt = [[big.tile([128, 456], F16, tag=f"x_{k}_{c}",
                            name=f"x_{k}_{c}", bufs=2)
                   for c in range(CHUNKS)] for k in range(8)]
            for c in range(CHUNKS):
                for k in range(8):
                    nc.sync.dma_start(xt[k][c][:],
                                      d_x.ap()[c, 128 * k:128 * (k + 1), :])
            nc.sync.dma_start(gd_t[:], d_gd.ap()[:])

            # -------- conv_gs: 8 non-center taps packed into M=128, k-major
            def setA_wave(wave):
                ps = {pr: ppool.tile([128, 1024], F32, tag="mm", name="mm")
                      for pr in wave}
                for k in range(8):
                    for pr in wave:
                        for i, c in enumerate(pr):
                            nc.tensor.matmul(
                                ps[pr][:, 512 * i:512 * i + 456], wA8[k][:],
                                xt[k][c][:], start=(k == 0), stop=(k == 7))
                for pr in wave:
                    nc.scalar.copy(
                        yA[:, chunk_base(pr[0]):chunk_base(pr[0]) + 456 * len(pr)]
                            .rearrange("p (b q) -> p b q", q=456),
                        ps_regions(ps[pr], len(pr)))

            # selector + center tap: g1 = relu(sum_t y[q+dt] + W_c.x + bgs)
            def selector_pair(pr):
                pg = ppool.tile([16, 1024], F32, tag="mm", name="mm")
                for i, c in enumerate(pr):
                    qb = chunk_base(c)
                    o = 512 * i
                    for ti, t in enumerate(NCTAPS):
                        nc.tensor.matmul(
                            pg[:, o:o + 456], sel8[ti][:],
                            yA[:, qb + DELTA[t]:qb + DELTA[t] + 456],
                            start=(ti == 0), stop=False)
                    for k in range(8):
                        nc.tensor.matmul(pg[:, o:o + 456], wct[k][:],
                                         xt[k][c][:],
                                         start=False, stop=(k == 7))
                nc.scalar.activation(
                    g1[:, 448 * pr[0]:448 * pr[0] + 448 * len(pr)]
                        .rearrange("p (b r w) -> p b r w", b=len(pr), w=56),
                    ps_interior(pg, len(pr)), AF.Relu, bias=bgs[:])

            # wave1 -> selector(P0) -> wave2 -> selector(P1..P3): gives the PE
            # x-load-independent work right after the sample boundary
            setA_wave(PAIRS[0:2])
            selector_pair(PAIRS[0])
            setA_wave(PAIRS[2:4])
            for pr in PAIRS[1:]:
                selector_pair(pr)

            # -------- conv1 first pairs' matmuls (fills PE during pooling)
            def conv1_mms(pr, p):
                ps = ppool.tile([128, 1024], F32, tag="mm", name="mm")
                for i, c in enumerate(pr):
                    for j in range(4):
                        nc.tensor.matmul(
                            ps[:, 512 * i:512 * i + 456], w1[p][j][:],
                            xt[4 * p + j][c][:],
                            start=(j == 0), stop=(j == 3))
                return ps

            conv1_ps = [(PAIRS[0], p, conv1_mms(PAIRS[0], p)) for p in range(2)]
            conv1_ps.append((PAIRS[1], 0, conv1_mms(PAIRS[1], 0)))

            # -------- pooling + mask decision + mask expansion
            nc.vector.reduce_sum(
                p1tmp[:].rearrange("p (r j) -> p r j", j=7),
                g1[:].rearrange("p (r c) -> p r c", c=56)
                     .rearrange("p r (j c) -> p r j c", c=8),
                axis=mybir.AxisListType.X)
            nc.vector.reduce_sum(
                pooled[:].rearrange("p (a j) -> p a j", j=7),
                p1tmp[:].rearrange("p (a r j) -> p a j r", r=8, j=7),
                axis=mybir.AxisListType.X)
            pm = ppool.tile([16, 49], F32, tag="mm", name="pm")
            nc.tensor.matmul(pm[0:4, 0:49], wd4[:], pooled[:], start=True, stop=True)
            nc.vector.tensor_tensor(s_t[:], pm[0:4, 0:49], gd_t[:], op=ALU.add)
            nc.vector.tensor_scalar(mask[:], s_t[:], 0.0, None, op0=ALU.is_gt)
            nc.sync.dma_start(d_mask.ap()[:], mask[:])
            for cc in range(2):
                pe = ppool.tile([128, 49], F32, tag="mm", name="pm")
                nc.tensor.matmul(pe[:, 0:49], Emat[cc][:], mask[:],
                                 start=True, stop=True)
                nc.scalar.copy(mexp[cc][:], pe[:, 0:49])

            # -------- per-pair stages (skewed pipeline below)
            def evict_conv1(pr, p, ps):
                n = 456 * len(pr)
                qb = chunk_base(pr[0])
                dst = (a1[p][:, qb:qb + n]
                       .rearrange("p (b r w) -> p b r w", b=len(pr), w=57)
                       [:, :, :, 0:56])
                nc.scalar.activation(dst, ps_interior(ps, len(pr)),
                                     AF.Relu, bias=b1[p][:])
                for i, c in enumerate(pr):
                    d4 = interior(a1[p][:, chunk_base(c):chunk_base(c) + 456])
                    d4 = d4.rearrange("p r (j c) -> p r j c", c=8)
                    nc.vector.tensor_tensor(d4, d4, mask_bcast(p, c), op=ALU.mult)

            def conv3_half(pr, ms):
                for m in ms:
                    ps = ppool.tile([128, 1024], F32, tag="mm", name="mm")
                    for i, c in enumerate(pr):
                        nc.tensor.matmul(ps[:, 512 * i:512 * i + 456], w3[m][:],
                                         z2[m // 4][:, 456 * c:456 * c + 456],
                                         start=True, stop=False)
                        nc.tensor.matmul(ps[:, 512 * i:512 * i + 456], ident[:],
                                         xt[m][c][:],
                                         start=False, stop=True)
                    st = spool.tile([128, 896], F32, tag="stage", name="stage")
                    sv = (st[:, 0:448 * len(pr)]
                          .rearrange("p (b r w) -> p b r w", b=len(pr), w=56))
                    if m % 2 == 0:
                        nc.scalar.activation(sv, ps_interior(ps, len(pr)), AF.Relu,
                                             bias=b3[m][:])
                    else:
                        nc.vector.tensor_scalar(sv, ps_interior(ps, len(pr)),
                                                b3[m][:], 0.0,
                                                op0=ALU.add, op1=ALU.max)
                    # stores go out the ACT HWDGE ring so next-sample x loads
                    # (sync ring) aren't queued behind them
                    nc.scalar.dma_start(
                        d_out.ap()[128 * m:128 * (m + 1),
                                   448 * pr[0]:448 * pr[0] + 448 * len(pr)],
                        st[:, 0:448 * len(pr)])

            def conv23_pair(pr):
                # conv3 halves follow the conv2 pairset that feeds them, so
                # out-stores spread across the conv2 phase
                n = 456 * len(pr)
                for p in range(2):
                    ps = ppool.tile([128, 1024], F32, tag="mm", name="mm")
                    for i, c in enumerate(pr):
                        qb = chunk_base(c)
                        for t in range(9):
                            nc.tensor.matmul(
                                ps[:, 512 * i:512 * i + 456], w2[p][t][:],
                                a1[p][:, qb + DELTA[t]:qb + DELTA[t] + 456],
                                start=(t == 0), stop=(t == 8))
                    zdst = (z2[p][:, 456 * pr[0]:456 * pr[0] + n]
                            .rearrange("p (b r w) -> p b r w", b=len(pr), w=57)
                            [:, :, :, 0:56])
                    nc.scalar.activation(zdst, ps_interior(ps, len(pr)),
                                         AF.Relu, bias=b2[p][:])
                    for i, c in enumerate(pr):
                        d4 = interior(z2[p][:, 456 * c:456 * c + 456])
                        d4 = d4.rearrange("p r (j c) -> p r j c", c=8)
                        nc.vector.tensor_tensor(d4, d4, mask_bcast(p, c),
                                                op=ALU.mult)
                    conv3_half(pr, range(4 * p, 4 * p + 4))

            # skewed pipeline: conv1 one pair ahead; conv3 follows conv2 so
            # stores + x-col releases spread across the sample
            for pr, p, ps in conv1_ps:
                evict_conv1(pr, p, ps)
            conv1_ps = None
            evict_conv1(PAIRS[1], 1, conv1_mms(PAIRS[1], 1))
            for pi in range(4):
                if pi + 2 < 4:
                    for p in range(2):
                        evict_conv1(PAIRS[pi + 2], p, conv1_mms(PAIRS[pi + 2], p))
                conv23_pair(PAIRS[pi])

    nc.compile()
    return nc


def _run(shared, percore, trace=False, tmpdir=None):
    from concourse.bass_utils import run_bass_kernel_spmd
    global _PROG
    if _PROG is None:
        _PROG = _build_program()
    in_maps = [dict(shared, **percore[s]) for s in range(B)]
    res = run_bass_kernel_spmd(_PROG, in_maps, core_ids=list(range(B)),
                               trace=trace, tmpdir=tmpdir)
    out = np.stack([res.results[s]["out"].reshape(PLANES, H, W) for s in range(B)])
    mask = np.stack([res.results[s]["mask_out"].reshape(4, MS, MS) for s in range(B)])
    return (out.astype(np.float32), mask.astype(np.float32)), res


def kernel(**inputs):
    shared, percore = _host_prep(inputs)
    (out, mask), _ = _run(shared, percore)
    return out, mask


# revision 24
# speedup vs baseline: 1.0329x; 1.0329x over previous
"""Trainium2 Bass kernel for nn_Bottleneck_refine (masked grouped bottleneck).

Sharding: data-parallel over batch B=8 across 8 NeuronCores (1 sample/core).

Math factorization (validated vs reference):
  - All eval-mode BNs folded into conv weights/biases on host.
  - mask = (wd/64 . pooled_sum + bd + gumbel_diff) > 0   (hard gumbel forward
    value is exactly the one-hot argmax; softmax is monotone).
  - x*mask1 is algebraically removable: conv1 is 1x1 grouped and the mask2
    multiply after relu(bn1(.)) re-zeroes exactly the masked cells.
  - Spatial layout on chip: flat padded rows of width 57 (56 cols + one zero
    separator) x 58 rows (zero top/bottom rows) + 1 leading guard col, so a
    3x3 conv tap (dh,dw) is a flat column shift dh*57+dw.
  - conv_gs (1024->16, 3x3) is done as two tap-packed matmul passes
    (M = taps x 16 out-ch: 5 taps then 4 taps) producing per-tap partials,
    then 9 selector-matmuls accumulate the shifted tap-sum in PSUM.
  - conv1/residual-identity in float32r (full-rate fp32 matmul), conv2/conv3
    and selector passes in bf16 (intermediates only; residual path stays f32).
"""

import os
import sys

import numpy as np

for _p in ("/opt/trn_rl_repo",):
    if os.path.isdir(_p) and _p not in sys.path:
        sys.path.insert(0, _p)

import ml_dtypes  # noqa: E402

BF16 = ml_dtypes.bfloat16

# problem constants
B = 8
CIN = 1024
H = W = 56
G = 4
MID = 256
PLANES = 1024
MS = 7
EPS = 1e-5

# padded flat spatial layout
WP = 57            # 56 data cols + 1 zero separator
HP = 58            # zero row 0, data rows 1..56, zero row 57
NPAD = HP * WP     # 3306
BUFW = NPAD + 2    # guard cols at both ends (flat index -1 / +NPAD of tap shifts)
NPIX = H * W       # 3136

TAPS = [(dh, dw) for dh in (-1, 0, 1) for dw in (-1, 0, 1)]
DELTA = [dh * WP + dw for dh, dw in TAPS]
NCTAPS = [0, 1, 2, 3, 5, 6, 7, 8]   # non-center taps (center=4 applied on x)
PAIRS = [(0, 1), (2, 3), (4, 5), (6,)]

NCHUNK_A = [472, 472, 472, 472, 472, 472, 474]   # setA passes cover [0, 3306)
CHUNKS = 7                                        # interior chunks of 8 rows


def _fold(w, g, b, m, v):
    s = (np.asarray(g, np.float32) / np.sqrt(np.asarray(v, np.float32) + EPS))
    w = np.asarray(w, np.float32)
    return (w * s[:, None, None, None]).astype(np.float32), \
        (np.asarray(b, np.float32) - np.asarray(m, np.float32) * s).astype(np.float32)


def _host_prep(inputs):
    """Build all host-side arrays (per-core and shared)."""
    x = np.asarray(inputs["x"], np.float32)
    wgs, bgs = _fold(inputs["conv_gs_w"], inputs["bn_gs_g"], inputs["bn_gs_b"],
                     inputs["bn_gs_m"], inputs["bn_gs_v"])
    w1, b1 = _fold(inputs["conv1_w"], inputs["bn1_g"], inputs["bn1_b"],
                   inputs["bn1_m"], inputs["bn1_v"])
    w2, b2 = _fold(inputs["conv2_w"], inputs["bn2_g"], inputs["bn2_b"],
                   inputs["bn2_m"], inputs["bn2_v"])
    w3, b3 = _fold(inputs["conv3_w"], inputs["bn3_g"], inputs["bn3_b"],
                   inputs["bn3_m"], inputs["bn3_v"])

    # ---- x: fp16 cell-aligned column tiles [7, CIN, 456] per sample
    # (image rows 8c..8c+7 with one zero separator col per row)
    xp = np.zeros((B, CIN, 56, WP), np.float16)
    xp[:, :, :, 0:56] = x.astype(np.float16)
    xcol = np.transpose(xp.reshape(B, CIN, 7, 456), (0, 2, 1, 3)).copy()

    # ---- conv_gs tap-packed lhsT:  wA1 [8,128,80], wA2 [8,128,64]
    # col ti*16+oc of set s = tap SETs[ti], out-ch oc; rows = global in-ch.
    wgs9 = wgs.reshape(16, 256, 9)                      # [oc, icl, tap]
    wA = np.zeros((8, 128, 9, 16), np.float32)          # [kchunk, r, tap, oc]
    for k in range(8):
        ic = 128 * k + np.arange(128)
        for oc in range(16):
            m = (ic // 256) == (oc // 4)
            wA[k, m, :, oc] = wgs9[oc, ic[m] % 256, :]
    # taps 0..3,5..8 packed into M=128 (center tap 4 applied directly on x)
    wA8 = wA[:, :, NCTAPS, :].reshape(8, 128, 128).copy()
    wct = wA[:, :, 4, :].copy()                          # [8,128,16]

    # ---- selector lhsT: sel8[ti] picks tap NCTAPS[ti]'s 16 rows
    sel8 = np.zeros((8, 128, 16), np.float32)
    for ti in range(8):
        sel8[ti, ti * 16 + np.arange(16), np.arange(16)] = 1.0

    # ---- conv1 lhsT [2,4,128,128] (pairset p, kchunk j): rows = in-ch
    # 512p+128j+r, cols mm = out-ch 128p+mm; nonzero iff j//2 == mm//64.
    w1m = w1[:, :, 0, 0]                                # [256 out, 256 in-local]
    w1T = np.zeros((2, 4, 128, 128), np.float32)
    for p in range(2):
        for j in range(4):
            for mm in range(128):
                if j // 2 != mm // 64:
                    continue
                w1T[p, j, :, mm] = w1m[128 * p + mm, 128 * (j % 2):128 * (j % 2) + 128]

    # ---- conv2 lhsT [2,9,128,128] bf16 (pairset p, tap t); block-diag 64x64
    w29 = w2.reshape(256, 64, 9)
    w2T = np.zeros((2, 9, 128, 128), np.float32)
    for p in range(2):
        for mm in range(128):
            base = (mm // 64) * 64
            w2T[p, :, base:base + 64, mm] = w29[128 * p + mm].T
    # ---- conv3 lhsT [8,128,128] bf16 (mchunk m8): rows = z2-chunk partition
    # (only the group's 64 rows are nonzero, so base partitions match)
    w3m = w3[:, :, 0, 0]                                # [1024, 64]
    w3T = np.zeros((8, 128, 128), np.float32)
    for m8 in range(8):
        r0 = ((m8 // 2) % 2) * 64
        w3T[m8, r0:r0 + 64, :] = w3m.reshape(8, 128, 64)[m8].T

    ident = np.eye(128, dtype=np.float32)

    # ---- mask decision
    fcw = np.asarray(inputs["fc_gs_w"], np.float32)[:, :, 0, 0]  # [8,4]
    fcb = np.asarray(inputs["fc_gs_b"], np.float32)              # [8]
    L = np.zeros((16, 8), np.float32)
    for c in range(8):
        gconv = c // 2
        L[gconv * 4:(gconv + 1) * 4, c] = fcw[c]
    wd4 = np.zeros((16, 4), np.float32)
    bd = np.zeros(4, np.float32)
    for g in range(4):
        wd4[:, g] = (L[:, 4 + g] - L[:, g]) / 64.0
        bd[g] = fcb[4 + g] - fcb[g]
    gn = np.asarray(inputs["gumbel_noise"], np.float32)
    gd = (gn[:, 1] - gn[:, 0]).reshape(B, 4, 49) + bd[None, :, None]  # [B,4,49]

    # ---- mask expand selector E [2,4,128]
    Emat = np.zeros((2, 4, 128), np.float32)
    for cc in range(2):
        c = np.arange(128)
        Emat[cc, 2 * cc + c // 64, c] = 1.0

    F16 = np.float16
    shared = {
        "wA8": wA8.astype(F16), "wct": wct.astype(F16),
        "sel8": sel8.astype(F16),
        "w1T": w1T.astype(F16),
        "w2T": w2T.astype(F16),
        "w3T": w3T.astype(F16),
        "ident": ident.astype(F16),
        "Emat": Emat,
        "wd4": wd4,
        "bgs": bgs.reshape(16, 1),
        "b1": b1.reshape(2, 128, 1),
        "b2": b2.reshape(2, 128, 1),
        "b3": b3.reshape(8, 128, 1),
    }
    percore = [{"xcol": xcol[s], "gd": gd[s]} for s in range(B)]
    return shared, percore


# ---------------------------------------------------------------------------
# Bass program
# ---------------------------------------------------------------------------

_PROG = None


def _build_program(nsamp=B):
    from contextlib import ExitStack

    import concourse.bacc as bacc
    import concourse.mybir as mybir
    import concourse.tile as tile

    dt = mybir.dt
    F32, F16 = dt.float32, dt.float16
    AF = mybir.ActivationFunctionType
    ALU = mybir.AluOpType

    nc = bacc.Bacc("TRN2", target_bir_lowering=False, debug=False)

    d_x = nc.dram_tensor("xcol", [7, CIN, 456], F16, kind="ExternalInput")
    d_gd = nc.dram_tensor("gd", [4, 49], F32, kind="ExternalInput")
    d_wA8 = nc.dram_tensor("wA8", [8, 128, 128], F16, kind="ExternalInput")
    d_wct = nc.dram_tensor("wct", [8, 128, 16], F16, kind="ExternalInput")
    d_sel8 = nc.dram_tensor("sel8", [8, 128, 16], F16, kind="ExternalInput")
    d_w1 = nc.dram_tensor("w1T", [2, 4, 128, 128], F16, kind="ExternalInput")
    d_w2 = nc.dram_tensor("w2T", [2, 9, 128, 128], F16, kind="ExternalInput")
    d_w3 = nc.dram_tensor("w3T", [8, 128, 128], F16, kind="ExternalInput")
    d_id = nc.dram_tensor("ident", [128, 128], F16, kind="ExternalInput")
    d_E = nc.dram_tensor("Emat", [2, 4, 128], F32, kind="ExternalInput")
    d_wd4 = nc.dram_tensor("wd4", [16, 4], F32, kind="ExternalInput")
    d_bgs = nc.dram_tensor("bgs", [16, 1], F32, kind="ExternalInput")
    d_b1 = nc.dram_tensor("b1", [2, 128, 1], F32, kind="ExternalInput")
    d_b2 = nc.dram_tensor("b2", [2, 128, 1], F32, kind="ExternalInput")
    d_b3 = nc.dram_tensor("b3", [8, 128, 1], F32, kind="ExternalInput")
    d_out = nc.dram_tensor("out", [PLANES, NPIX], F32, kind="ExternalOutput")
    d_mask = nc.dram_tensor("mask_out", [4, 49], F32, kind="ExternalOutput")

    def interior(ap, w=WP):
        """[P, n*w] contiguous slice -> [P, n, 56] strided interior view."""
        return ap.rearrange("p (r w) -> p r w", w=w)[:, :, 0:56]

    def ps_interior(ps, nb):
        """[128,1024] psum pair tile -> [P, nb, 8, 56] interior of bank regions."""
        return (ps[:, 0:512 * nb].rearrange("p (b q) -> p b q", q=512)
                [:, :, 0:456].rearrange("p b (r w) -> p b r w", w=57)[:, :, :, 0:56])

    def ps_regions(ps, nb):
        """[128,1024] psum pair tile -> [P, nb, 456] bank regions."""
        return (ps[:, 0:512 * nb].rearrange("p (b q) -> p b q", q=512)[:, :, 0:456])

    with tile.TileContext(nc) as tc, ExitStack() as ex:
        wpool = ex.enter_context(tc.tile_pool(name="w", bufs=1))
        big = ex.enter_context(tc.tile_pool(name="big", bufs=1))
        ppool = ex.enter_context(tc.tile_pool(name="ps", bufs=4, space="PSUM"))
        spool = ex.enter_context(tc.tile_pool(name="stage", bufs=8))

        def load(dram_ap, shape, dtyp, tag):
            t = wpool.tile(shape, dtyp, tag=tag, name=tag)
            nc.sync.dma_start(t[:], dram_ap)
            return t

        wA8 = [load(d_wA8.ap()[k], [128, 128], F16, f"wA8_{k}") for k in range(8)]
        wct = [load(d_wct.ap()[k], [128, 16], F16, f"wct_{k}") for k in range(8)]
        sel8 = [load(d_sel8.ap()[t], [128, 16], F16, f"sel8_{t}") for t in range(8)]
        w1 = [[load(d_w1.ap()[p, j], [128, 128], F16, f"w1_{p}_{j}")
               for j in range(4)] for p in range(2)]
        w2 = [[load(d_w2.ap()[p, t], [128, 128], F16, f"w2_{p}_{t}")
               for t in range(9)] for p in range(2)]
        w3 = [load(d_w3.ap()[m], [128, 128], F16, f"w3_{m}") for m in range(8)]
        ident = load(d_id.ap()[:], [128, 128], F16, "ident")
        Emat = [load(d_E.ap()[cc], [4, 128], F32, f"E_{cc}") for cc in range(2)]
        wd4 = load(d_wd4.ap()[:], [16, 4], F32, "wd4")
        bgs = load(d_bgs.ap()[:], [16, 1], F32, "bgs")
        b1 = [load(d_b1.ap()[p], [128, 1], F32, f"b1_{p}") for p in range(2)]
        b2 = [load(d_b2.ap()[p], [128, 1], F32, f"b2_{p}") for p in range(2)]
        b3 = [load(d_b3.ap()[m], [128, 1], F32, f"b3_{m}") for m in range(8)]

        a1 = [big.tile([128, BUFW], F16, tag=f"a1_{p}", name=f"a1_{p}") for p in range(2)]
        yA = big.tile([128, BUFW], F16, tag="yA", name="yA")
        z2 = [big.tile([128, 3192], F16, tag=f"z2_{p}", name=f"z2_{p}") for p in range(2)]
        g1 = big.tile([16, NPIX], F16, tag="g1", name="g1")
        p1tmp = big.tile([16, 392], F32, tag="p1tmp", name="p1tmp")
        pooled = big.tile([16, 49], F32, tag="pooled", name="pooled")
        gd_t = big.tile([4, 49], F32, tag="gd", name="gd")
        s_t = big.tile([4, 49], F32, tag="s_t", name="s_t")
        mask = big.tile([4, 49], F32, tag="mask", name="mask")
        mexp = [big.tile([128, 49], F16, tag=f"mexp_{cc}", name=f"mexp_{cc}") for cc in range(2)]

        # zero pad/guard structure once (interior writes preserve it)
        for t in a1 + [yA] + z2:
            nc.vector.memset(t[:], 0.0)

        def chunk_base(c):
            # first flat index (incl. guard offset) of interior chunk c's rows
            return 1 + WP * (1 + 8 * c)

        def mask_bcast(p, c):
            # mask for ch-chunk p at cell-row c, broadcast to [128, 8, 7, 8]
            return (mexp[p][:, 7 * c:7 * c + 7]
                    .unsqueeze(1).unsqueeze(3).broadcast_to([128, 8, 7, 8]))

        for s in range(nsamp):
            # double-buffered x tiles: next sample's loads have no WAR on this
            # sample's reads, so they prefetch across the whole sample
            xt = [[big.tile([128, 456], F16, tag=f"x_{k}_{c}",
                            name=f"x_{k}_{c}", bufs=2)
                   for c in range(CHUNKS)] for k in range(8)]
            for c in range(CHUNKS):
                for k in range(8):
                    nc.sync.dma_start(xt[k][c][:],
                                      d_x.ap()[c, 128 * k:128 * (k + 1), :])
            nc.sync.dma_start(gd_t[:], d_gd.ap()[:])

            # -------- conv_gs: 8 non-center taps packed into M=128, k-major
            def setA_wave(wave):
                ps = {pr: ppool.tile([128, 1024], F32, tag="mm", name="mm")
                      for pr in wave}
                for k in range(8):
                    for pr in wave:
                        for i, c in enumerate(pr):
                            nc.tensor.matmul(
                                ps[pr][:, 512 * i:512 * i + 456], wA8[k][:],
                                xt[k][c][:], start=(k == 0), stop=(k == 7))
                for pr in wave:
                    nc.scalar.copy(
                        yA[:, chunk_base(pr[0]):chunk_base(pr[0]) + 456 * len(pr)]
                            .rearrange("p (b q) -> p b q", q=456),
                        ps_regions(ps[pr], len(pr)))

            # selector + center tap: g1 = relu(sum_t y[q+dt] + W_c.x + bgs)
            def selector_pair(pr):
                pg = ppool.tile([16, 1024], F32, tag="mm", name="mm")
                for i, c in enumerate(pr):
                    qb = chunk_base(c)
                    o = 512 * i
                    for ti, t in enumerate(NCTAPS):
                        nc.tensor.matmul(
                            pg[:, o:o + 456], sel8[ti][:],
                            yA[:, qb + DELTA[t]:qb + DELTA[t] + 456],
                            start=(ti == 0), stop=False)
                    for k in range(8):
                        nc.tensor.matmul(pg[:, o:o + 456], wct[k][:],
                                         xt[k][c][:],
                                         start=False, stop=(k == 7))
                nc.scalar.activation(
                    g1[:, 448 * pr[0]:448 * pr[0] + 448 * len(pr)]
                        .rearrange("p (b r w) -> p b r w", b=len(pr), w=56),
                    ps_interior(pg, len(pr)), AF.Relu, bias=bgs[:])

            # wave1 -> selector(P0) -> wave2 -> selector(P1..P3): gives the PE
            # x-load-independent work right after the sample boundary
            setA_wave(PAIRS[0:2])
            selector_pair(PAIRS[0])
            setA_wave(PAIRS[2:4])
            for pr in PAIRS[1:]:
                selector_pair(pr)

            # -------- conv1 first pairs' matmuls (fills PE during pooling)
            def conv1_mms(pr, p):
                ps = ppool.tile([128, 1024], F32, tag="mm", name="mm")
                for i, c in enumerate(pr):
                    for j in range(4):
                        nc.tensor.matmul(
                            ps[:, 512 * i:512 * i + 456], w1[p][j][:],
                            xt[4 * p + j][c][:],
                            start=(j == 0), stop=(j == 3))
                return ps

            conv1_ps = [(PAIRS[0], p, conv1_mms(PAIRS[0], p)) for p in range(2)]
            conv1_ps.append((PAIRS[1], 0, conv1_mms(PAIRS[1], 0)))

            # -------- pooling + mask decision + mask expansion
            nc.vector.reduce_sum(
                p1tmp[:].rearrange("p (r j) -> p r j", j=7),
                g1[:].rearrange("p (r c) -> p r c", c=56)
                     .rearrange("p r (j c) -> p r j c", c=8),
                axis=mybir.AxisListType.X)
            nc.vector.reduce_sum(
                pooled[:].rearrange("p (a j) -> p a j", j=7),
                p1tmp[:].rearrange("p (a r j) -> p a j r", r=8, j=7),
                axis=mybir.AxisListType.X)
            pm = ppool.tile([16, 49], F32, tag="mm", name="pm")
            nc.tensor.matmul(pm[0:4, 0:49], wd4[:], pooled[:], start=True, stop=True)
            nc.vector.tensor_tensor(s_t[:], pm[0:4, 0:49], gd_t[:], op=ALU.add)
            nc.vector.tensor_scalar(mask[:], s_t[:], 0.0, None, op0=ALU.is_gt)
            nc.sync.dma_start(d_mask.ap()[:], mask[:])
            for cc in range(2):
                pe = ppool.tile([128, 49], F32, tag="mm", name="pm")
                nc.tensor.matmul(pe[:, 0:49], Emat[cc][:], mask[:],
                                 start=True, stop=True)
                nc.scalar.copy(mexp[cc][:], pe[:, 0:49])

            # -------- per-pair stages (skewed pipeline below)
            def evict_conv1(pr, p, ps):
                n = 456 * len(pr)
                qb = chunk_base(pr[0])
                dst = (a1[p][:, qb:qb + n]
                       .rearrange("p (b r w) -> p b r w", b=len(pr), w=57)
                       [:, :, :, 0:56])
                nc.scalar.activation(dst, ps_interior(ps, len(pr)),
                                     AF.Relu, bias=b1[p][:])
                for i, c in enumerate(pr):
                    d4 = interior(a1[p][:, chunk_base(c):chunk_base(c) + 456])
                    d4 = d4.rearrange("p r (j c) -> p r j c", c=8)
                    nc.vector.tensor_tensor(d4, d4, mask_bcast(p, c), op=ALU.mult)

            def conv3_half(pr, ms):
                for m in ms:
                    ps = ppool.tile([128, 1024], F32, tag="mm", name="mm")
                    for i, c in enumerate(pr):
                        nc.tensor.matmul(ps[:, 512 * i:512 * i + 456], w3[m][:],
                                         z2[m // 4][:, 456 * c:456 * c + 456],
                                         start=True, stop=False)
                        nc.tensor.matmul(ps[:, 512 * i:512 * i + 456], ident[:],
                                         xt[m][c][:],
                                         start=False, stop=True)
                    st = spool.tile([128, 896], F32, tag="stage", name="stage")
                    sv = (st[:, 0:448 * len(pr)]
                          .rearrange("p (b r w) -> p b r w", b=len(pr), w=56))
                    if m % 2 == 0:
                        nc.scalar.activation(sv, ps_interior(ps, len(pr)), AF.Relu,
                                             bias=b3[m][:])
                    else:
                        nc.vector.tensor_scalar(sv, ps_interior(ps, len(pr)),
                                                b3[m][:], 0.0,
                                                op0=ALU.add, op1=ALU.max)
                    # stores go out the ACT HWDGE ring so next-sample x loads
                    # (sync ring) aren't queued behind them
                    nc.scalar.dma_start(
                        d_out.ap()[128 * m:128 * (m + 1),
                                   448 * pr[0]:448 * pr[0] + 448 * len(pr)],
                        st[:, 0:448 * len(pr)])

            def conv23_pair(pr):
                # conv3 halves follow the conv2 pairset that feeds them, so
                # out-stores spread across the conv2 phase
                n = 456 * len(pr)
                for p in range(2):
                    ps = ppool.tile([128, 1024], F32, tag="mm", name="mm")
                    for i, c in enumerate(pr):
                        qb = chunk_base(c)
                        for t in range(9):
                            nc.tensor.matmul(
                                ps[:, 512 * i:512 * i + 456], w2[p][t][:],
                                a1[p][:, qb + DELTA[t]:qb + DELTA[t] + 456],
                                start=(t == 0), stop=(t == 8))
                    zdst = (z2[p][:, 456 * pr[0]:456 * pr[0] + n]
                            .rearrange("p (b r w) -> p b r w", b=len(pr), w=57)
                            [:, :, :, 0:56])
                    nc.scalar.activation(zdst, ps_interior(ps, len(pr)),
                                         AF.Relu, bias=b2[p][:])
                    for i, c in enumerate(pr):
                        d4 = interior(z2[p][:, 456 * c:456 * c + 456])
                        d4 = d4.rearrange("p r (j c) -> p r j c", c=8)
                        nc.vector.tensor_tensor(d4, d4, mask_bcast(p, c),
                                                op=ALU.mult)
                for ms in (range(0, 4), range(4, 8)):
                    conv3_half(pr, ms)

            # skewed pipeline: conv1 one pair ahead; conv3 follows conv2 so
            # stores + x-col releases spread across the sample
            for pr, p, ps in conv1_ps:
                evict_conv1(pr, p, ps)
            conv1_ps = None
            evict_conv1(PAIRS[1], 1, conv1_mms(PAIRS[1], 1))
            for pi in range(4):
                if pi + 2 < 4:
                    for p in range(2):
                        evict_conv1(PAIRS[pi + 2], p, conv1_mms(PAIRS[pi + 2], p))
                conv23_pair(PAIRS[pi])

    nc.compile()
    return nc


def _run(shared, percore, trace=False, tmpdir=None):
    from concourse.bass_utils import run_bass_kernel_spmd
    global _PROG
    if _PROG is None:
        _PROG = _build_program()
    in_maps = [dict(shared, **percore[s]) for s in range(B)]
    res = run_bass_kernel_spmd(_PROG, in_maps, core_ids=list(range(B)),
                               trace=trace, tmpdir=tmpdir)
    out = np.stack([res.results[s]["out"].reshape(PLANES, H, W) for s in range(B)])
    mask = np.stack([res.results[s]["mask_out"].reshape(4, MS, MS) for s in range(B)])
    return (out.astype(np.float32), mask.astype(np.float32)), res


def kernel(**inputs):
    shared, percore = _host_prep(inputs)
    (out, mask), _ = _run(shared, percore)
    return out, mask
